# revision 1
# baseline (speedup 1.0000x reference)
"""Trainium2 Bass kernel for ragged bag-attention (nn_Attention).

Algorithm (per sentence i, bag b): logit_i = <x_i, att[q_i]*rel[q_i]>;
w = softmax(logit) within bag; bag_repr_b = sum w_i x_i; out = bag_repr @ rel.T + bias.

Device strategy (8 cores, sentence-sharded):
  - Sentences packed into 128-row chunks with <=16 bag-fragments per chunk
    (bags may split across chunks/cores; fragment partial sums are combined
    on host, exploiting exp(logit) being max-free safe: |logit| < ~0.5).
  - Per chunk: G = onehotT(q).T @ (att*rel)  (PE, fp32r)
               logit = rowsum(x * G)         (DVE tensor_tensor_reduce)
               e = exp(logit + pad_bias)     (ACT)
               E^T[i,j] = (j==relseg_i)*e_i  (DVE tensor_scalar)
               frag_sums = E^T.T @ [x|1]     (PE, fp32r -> PSUM)
  - Every 8 chunks the PSUM fragment table [128, 691] is copied to SBUF (DVE)
    and DMA'd out. Host: U = table @ rel.T, bin by bag, divide by denom, +bias.
"""
import sys
sys.path.insert(0, '/opt/trn_rl_repo')
import numpy as np

NCORES = 8
DIM = 690
NCLS = 53
CHUNK = 128
BSLOT = 16
GROUP = 4           # chunks per PSUM flush group

_cache = {}         # nchunk -> compiled Bass module


def _pack_core(scope, seg, lo, hi):
    """Pack sentences [lo,hi) into chunks of <=CHUNK sentences and <=BSLOT
    bag-fragments. Returns list of chunks, each a list of (bag, start, take)."""
    b0, b1 = int(seg[lo]), int(seg[hi - 1])
    chunks, cur, fill = [], [], 0
    for b in range(b0, b1 + 1):
        s = max(int(scope[b]), lo)
        e = min(int(scope[b + 1]), hi)
        m = e - s
        while m > 0:
            if fill == CHUNK or len(cur) == BSLOT:
                chunks.append(cur)
                cur, fill = [], 0
            take = min(m, CHUNK - fill)
            cur.append((b, s, take))
            fill += take
            s += take
            m -= take
    if cur:
        chunks.append(cur)
    return chunks


def _build_module(nchunk):
    from concourse import bacc, mybir
    from concourse.tile import TileContext

    f32 = mybir.dt.float32
    f32r = mybir.dt.float32r
    S = nchunk * CHUNK
    W = DIM + 2          # 692 padded row width
    assert nchunk % 8 == 0
    groups = nchunk // GROUP

    nc = bacc.Bacc()
    bf16 = mybir.dt.bfloat16
    # xp is host-preblocked: row (tb*128+p) holds the 4 chunk-rows
    # {512tb+128u+p : u<4} side by side -> one 11KB descriptor per partition.
    xp_d = nc.declare_dram_parameter("xp", [(nchunk // 4) * CHUNK, 4 * W], f32r,
                                     isOutput=False)
    oh_d = nc.declare_dram_parameter("oh", [NCLS, S], bf16, isOutput=False)
    cw_d = nc.declare_dram_parameter("cw", [NCLS, DIM], bf16, isOutput=False)
    rs_d = nc.declare_dram_parameter("rs", [CHUNK, nchunk], f32, isOutput=False)
    io_d = nc.declare_dram_parameter("io32", [CHUNK, 2 * BSLOT], f32, isOutput=False)
    tab_d = nc.declare_dram_parameter("tab", [nchunk * BSLOT, W], f32,
                                      isOutput=True)

    with TileContext(nc) as tc:
        with (
            tc.tile_pool(name="consts", bufs=1) as cpool,
            tc.tile_pool(name="xb", bufs=4) as xpool,
            tc.tile_pool(name="prod", bufs=2) as ppool,
            tc.tile_pool(name="small", bufs=4) as spool,
            tc.tile_pool(name="flush", bufs=2) as fpool,
            tc.tile_pool(name="gps", bufs=2, space="PSUM") as gpool,
            tc.tile_pool(name="bags", bufs=2, space="PSUM") as bpool,
        ):
            oh_sb = cpool.tile([NCLS, S], bf16)
            nc.scalar.dma_start(out=oh_sb[:, :], in_=oh_d[:, :])
            cw_sb = cpool.tile([NCLS, DIM], bf16)
            nc.scalar.dma_start(out=cw_sb[:, :], in_=cw_d[:, :])
            rs_sb = cpool.tile([CHUNK, nchunk], f32)
            nc.scalar.dma_start(out=rs_sb[:, :], in_=rs_d[:, :])
            io_sb = cpool.tile([CHUNK, 2 * BSLOT], f32)
            nc.scalar.dma_start(out=io_sb[:, :], in_=io_d[:, :])

            fl = None
            for tb in range(nchunk // 4):
                # one DMA loads 4 chunks: DRAM rows (u p) -> SBUF [p, u*W:(u+1)*W]
                xb = xpool.tile([CHUNK, 4 * W], f32r)
                nc.sync.dma_start(
                    out=xb[:, :],
                    in_=xp_d[tb * CHUNK:(tb + 1) * CHUNK, :])
                for u4 in range(4):
                    t = tb * 4 + u4
                    xe = xb[:, u4 * W:(u4 + 1) * W]
                    if t % 2 == 0:
                        bag = bpool.tile([32, 1024], f32)  # [0:346],[512:858]

                    G = gpool.tile([CHUNK, 1024], f32)    # [0:346],[512:856]
                    ohT = oh_sb[:, t * CHUNK:(t + 1) * CHUNK]
                    nc.tensor.matmul(G[:, 0:346], ohT, cw_sb[:, 0:346],
                                     start=True, stop=True)
                    nc.tensor.matmul(G[:, 512:856], ohT, cw_sb[:, 346:DIM],
                                     start=True, stop=True)

                    prod = ppool.tile([CHUNK, DIM], f32)
                    la = spool.tile([CHUNK, 1], f32)
                    lb2 = spool.tile([CHUNK, 1], f32)
                    xv = xe.bitcast(f32)
                    nc.vector.affine_mul_reduce(
                        out=prod[:, 0:346], accum_out=la[:, 0:1],
                        in0=xv[:, 0:346], in1=G[:, 0:346], scale=1.0, bias=0.0)
                    nc.vector.affine_mul_reduce(
                        out=prod[:, 346:DIM], accum_out=lb2[:, 0:1],
                        in0=xv[:, 346:DIM], in1=G[:, 512:856], scale=1.0, bias=0.0)

                    # e = exp(la + lb2); pad rows are all-zero in xe (incl the
                    # ones column) so their e value is irrelevant.
                    e = spool.tile([CHUNK, 1], f32)
                    nc.scalar.activation(e[:, 0:1], la[:, 0:1],
                                         mybir.ActivationFunctionType.Exp,
                                         bias=lb2[:, 0:1], scale=1.0)

                    # two consecutive chunks share one 32-row PSUM block:
                    # even chunk slots 0:16, odd chunk slots 16:32 (host adds
                    # 16 to relseg of odd chunks), accumulated via start/stop.
                    ET = spool.tile([CHUNK, 2 * BSLOT], f32r)
                    nc.vector.tensor_scalar(
                        out=ET[:, :], in0=io_sb[:, :], scalar1=rs_sb[:, t:t + 1],
                        scalar2=e[:, 0:1], op0=mybir.AluOpType.is_equal,
                        op1=mybir.AluOpType.mult)

                    first = (t % 2 == 0)
                    nc.tensor.matmul(bag[0:32, 0:346], ET[:, :], xe[:, 0:346],
                                     start=first, stop=not first)
                    nc.tensor.matmul(bag[0:32, 512:858], ET[:, :],
                                     xe[:, 346:W], start=first, stop=not first)

                    if t % 2 == 1:
                        p = t // 2
                        if p % 4 == 0:
                            fl = fpool.tile([32, 4 * W], f32)
                        # one copy per pair: both PSUM banks via 3D AP
                        nc.scalar.copy(
                            out=fl[:, (p % 4) * W:(p % 4) * W + 692]
                                .rearrange("q (a b) -> q a b", a=2, b=346),
                            in_=bag[0:32, 0:1024]
                                .rearrange("q (a b) -> q a b", a=2, b=512)
                                [:, :, 0:346])
                        if p % 4 == 3:
                            q4 = p // 4
                            dst = tab_d[q4 * 4 * 32:(q4 + 1) * 4 * 32, :]
                            nc.scalar.dma_start(
                                out=dst.rearrange("(u q) d -> q u d", u=4),
                                in_=fl[:, :].rearrange("q (u d) -> q u d", u=4))

    nc.compile()
    return nc


def _prepare(x, rel_weight, att_weight, bias, attention_query, scope):
    x = np.asarray(x, dtype=np.float32)
    rel_weight = np.asarray(rel_weight, dtype=np.float32)
    att_weight = np.asarray(att_weight, dtype=np.float32)
    bias = np.asarray(bias, dtype=np.float32)
    q = np.asarray(attention_query).astype(np.int64)
    scope = np.asarray(scope).astype(np.int64)

    nsent = x.shape[0]
    nbags = len(scope) - 1
    score = nsent // NCORES
    seg = (np.searchsorted(scope, np.arange(nsent), side='right') - 1)
    import ml_dtypes
    cw = (att_weight * rel_weight).astype(ml_dtypes.bfloat16)

    all_chunks = [_pack_core(scope, seg, c * score, (c + 1) * score)
                  for c in range(NCORES)]
    nchunk = max(len(ch) for ch in all_chunks)
    nchunk = (nchunk + 7) // 8 * 8      # device loop needs a multiple of 8
    S = nchunk * CHUNK

    import ml_dtypes
    iota32 = np.ascontiguousarray(
        np.broadcast_to(np.arange(2 * BSLOT, dtype=np.float32), (CHUNK, 2 * BSLOT)))
    in_maps = []
    frag2bag = []
    for c in range(NCORES):
        idx = np.full(S, -1, np.int64)
        relseg = np.zeros(S, np.float32)
        f2b = np.full((nchunk, BSLOT), -1, np.int64)
        for k, ch in enumerate(all_chunks[c]):
            p = k * CHUNK
            for j, (b, s, take) in enumerate(ch):
                idx[p:p + take] = np.arange(s, s + take)
                relseg[p:p + take] = j + BSLOT * (k % 2)
                f2b[k, j] = b
                p += take
        valid = idx >= 0
        xp = np.zeros((S, DIM + 2), np.float32)
        xp[valid, DIM] = 1.0
        xp[valid, :DIM] = x[idx[valid]]
        # pre-block: [nblocks, 4, 128, W] -> [nblocks, 128, 4, W] flat
        xp = np.ascontiguousarray(
            xp.reshape(nchunk // 4, 4, CHUNK, DIM + 2).transpose(0, 2, 1, 3)
        ).reshape((nchunk // 4) * CHUNK, 4 * (DIM + 2))
        qp = np.zeros(S, np.int64)
        qp[valid] = q[idx[valid]]
        oh = (qp[None, :] == np.arange(NCLS)[:, None]).astype(ml_dtypes.bfloat16)
        in_maps.append({
            "xp": xp,
            "oh": np.ascontiguousarray(oh),
            "cw": cw,
            "rs": np.ascontiguousarray(relseg.reshape(nchunk, CHUNK).T),
            "io32": iota32,
        })
        frag2bag.append(f2b)
    return in_maps, frag2bag, nchunk, nbags, rel_weight, bias


def _assemble(tables, frag2bag, nchunk, nbags, rel_weight, bias):
    num = np.zeros((nbags, NCLS))
    den = np.zeros(nbags)
    for c in range(NCORES):
        table = np.asarray(tables[c], dtype=np.float32).reshape(
            nchunk * BSLOT, DIM + 2)
        U = table[:, :DIM] @ rel_weight.T
        d = table[:, DIM]
        fb = frag2bag[c].ravel()
        v = fb >= 0
        for k in range(NCLS):
            num[:, k] += np.bincount(fb[v], U[v, k], minlength=nbags)
        den += np.bincount(fb[v], d[v], minlength=nbags)
    return (num / den[:, None] + bias[None, :]).astype(np.float32)


def kernel(x, rel_weight, att_weight, bias, attention_query, scope):
    from concourse.bass_utils import run_bass_kernel_spmd

    in_maps, frag2bag, nchunk, nbags, rel, b = _prepare(
        x, rel_weight, att_weight, bias, attention_query, scope)
    if nchunk not in _cache:
        _cache[nchunk] = _build_module(nchunk)
    nc = _cache[nchunk]
    res = run_bass_kernel_spmd(nc, in_maps, list(range(NCORES)))
    tables = [res.results[c]["tab"] for c in range(NCORES)]
    return _assemble(tables, frag2bag, nchunk, nbags, rel, b)



# revision 13
# speedup vs baseline: 2.0748x; 2.0748x over previous
"""Trainium2 Bass kernel for ragged bag-attention (nn_Attention).

Algorithm (per sentence i in bag b): logit_i = <x_i, att[q_i]*rel[q_i]>;
w = softmax(logit) within bag; out[b] = (sum_i w_i x_i) @ rel.T + bias.

Device strategy (8 cores, sentence-sharded, x shipped TRANSPOSED):
  - x rows packed into 128-row chunks; groups of GCH chunks share a PSUM
    accumulator with SLOTS bag-slots (bags may split across groups/cores;
    partial sums combined on host - exp(logit) is max-free safe, |logit|<~1).
  - x is sent d-major: 6 dtiles of [115, 128] per chunk. PE computes, with
    the x-dtile as STATIONARY, PY[s,0:106] = x_s @ [cw|rel].T  (cw=att*rel),
    accumulated over 6 dtiles -> all-class logits P and rel-projection Y in
    one pass, only 106 moving cols/dtile.
  - logit = rowsum(onehot(q) * P) on DVE (53-wide affine_mul_reduce).
  - ET[s,f] = exp(logit_s)*[slot_s==f] built by ONE ACT op:
    exp(50*IND2 + logit) with IND2 = [slot==f]-1 in {-1,0} from Pool.
  - bag accum: PSUM[f,0:54] += ET.T @ [Y | 1]  (54 moving cols), flushed
    to SBUF every GCH chunks, one final DMA of the tiny U-table.
  - Host: num[bag] += U[slot,0:53], den[bag] += U[slot,53];
    out = num/den + bias.
"""
import sys
sys.path.insert(0, '/opt/trn_rl_repo')
import numpy as np

NCORES = 8
DIM = 690
NCLS = 53
CHUNK = 128
DW = 115            # dims per dtile (6*115 = 690)
XR = DW + 1         # x-tile rows: 115 data + a constant-ones row
NDT = 6
MOV = 2 * NCLS + 1  # 107 moving cols: [cw | rel | unit] -> PY = [P | Y | 1]
SLOTS = 64          # bag slots per PSUM group
GCH = 10            # chunks per PSUM group
DMAB = 10           # chunks per input DMA batch
LAG = 4             # chunks between weight-build and bag matmul

_cache = {}         # nchunk -> compiled Bass module


def _pack_core(scope, seg, lo, hi):
    """Pack sentences [lo,hi) into 128-row chunks; groups of GCH chunks may
    hold at most SLOTS distinct bags (pad to group end when exceeded).
    Returns (rows, slots, f2b): sentence idx per row (-1 pad), slot per row,
    and per-group {bag: slot} maps."""
    group_rows = GCH * CHUNK
    rows, slots, f2b = [], [], []
    cur = None
    b0, b1 = int(seg[lo]), int(seg[hi - 1])
    for b in range(b0, b1 + 1):
        s = max(int(scope[b]), lo)
        e = min(int(scope[b + 1]), hi)
        while s < e:
            if len(rows) % group_rows == 0:
                cur = {}
                f2b.append(cur)
            gend = (len(rows) // group_rows + 1) * group_rows
            if b not in cur:
                if len(cur) == SLOTS:
                    pad = gend - len(rows)
                    rows.extend([-1] * pad)
                    slots.extend([-1] * pad)
                    continue
                cur[b] = len(cur)
            sl = cur[b]
            take = min(e - s, gend - len(rows))
            rows.extend(range(s, s + take))
            slots.extend([sl] * take)
            s += take
    return rows, slots, f2b


def _build_module(nchunk):
    from concourse import bacc, mybir
    from concourse.tile import TileContext

    f32 = mybir.dt.float32
    bf16 = mybir.dt.bfloat16
    eq = mybir.AluOpType.is_equal
    mult = mybir.AluOpType.mult
    sub = mybir.AluOpType.subtract
    ngroups = nchunk // GCH
    assert nchunk % DMAB == 0 and nchunk % GCH == 0

    nc = bacc.Bacc()
    xt_d = nc.declare_dram_parameter("xt", [XR, nchunk * NDT * CHUNK], bf16,
                                     isOutput=False)
    qi_d = nc.declare_dram_parameter("qi", [CHUNK, nchunk], f32, isOutput=False)
    si_d = nc.declare_dram_parameter("si", [CHUNK, nchunk], f32, isOutput=False)
    io_d = nc.declare_dram_parameter("io", [CHUNK, SLOTS], bf16, isOutput=False)
    cw_d = nc.declare_dram_parameter("cwrel", [XR, NDT * MOV], bf16,
                                     isOutput=False)
    ut_d = nc.declare_dram_parameter("ut", [SLOTS, ngroups * 54], f32,
                                     isOutput=True)

    with TileContext(nc) as tc:
        with (
            tc.tile_pool(name="consts", bufs=1) as cpool,
            tc.tile_pool(name="xb", bufs=3) as xpool,
            tc.tile_pool(name="oh", bufs=4) as ohpool,
            tc.tile_pool(name="ind", bufs=4) as indpool,
            tc.tile_pool(name="scr", bufs=2) as scrpool,
            tc.tile_pool(name="lg", bufs=4) as lgpool,
            tc.tile_pool(name="y", bufs=LAG + 2) as ypool,
            tc.tile_pool(name="et", bufs=LAG + 2) as etpool,
            tc.tile_pool(name="py", bufs=4, space="PSUM") as pypool,
            tc.tile_pool(name="bag", bufs=2, space="PSUM") as bagpool,
        ):
            qi_sb = cpool.tile([CHUNK, nchunk], f32)
            nc.scalar.dma_start(out=qi_sb[:, :], in_=qi_d[:, :])
            si_sb = cpool.tile([CHUNK, nchunk], f32)
            nc.scalar.dma_start(out=si_sb[:, :], in_=si_d[:, :])
            io_sb = cpool.tile([CHUNK, SLOTS], bf16)
            nc.scalar.dma_start(out=io_sb[:, :], in_=io_d[:, :])
            cw_sb = cpool.tile([XR, NDT * MOV], bf16)
            nc.scalar.dma_start(out=cw_sb[:, :], in_=cw_d[:, :])
            ut_sb = cpool.tile([SLOTS, ngroups * 54], f32)

            ets, ys, bag = {}, {}, None

            def emit_bag(t2):
                nonlocal bag
                g, u = t2 // GCH, t2 % GCH
                if u == 0:
                    bag = bagpool.tile([SLOTS, 54], f32)
                nc.tensor.matmul(bag[:, :], ets[t2], ys[t2][:, :],
                                 start=(u == 0), stop=(u == GCH - 1))
                del ets[t2], ys[t2]
                if u == GCH - 1:
                    nc.scalar.copy(out=ut_sb[:, g * 54:(g + 1) * 54],
                                   in_=bag[:, :])

            xb = None
            for t in range(nchunk):
                if t % DMAB == 0:
                    xb = xpool.tile([XR, DMAB * NDT * CHUNK], bf16)
                    nc.sync.dma_start(
                        out=xb[:, :],
                        in_=xt_d[:, t * NDT * CHUNK:(t + DMAB) * NDT * CHUNK])
                xe = xb[:, (t % DMAB) * NDT * CHUNK:]

                py = pypool.tile([CHUNK, MOV], f32)
                for j in range(NDT):
                    nc.tensor.matmul(
                        py[:, :], xe[:, j * CHUNK:(j + 1) * CHUNK],
                        cw_sb[:, j * MOV:(j + 1) * MOV],
                        start=(j == 0), stop=(j == NDT - 1))

                oht = ohpool.tile([CHUNK, NCLS], bf16)
                nc.gpsimd.tensor_scalar(
                    out=oht[:, :], in0=io_sb[:, 0:NCLS],
                    scalar1=qi_sb[:, t:t + 1], scalar2=1.0, op0=eq, op1=mult)
                ind = indpool.tile([CHUNK, SLOTS], bf16)
                nc.gpsimd.tensor_scalar(
                    out=ind[:, :], in0=io_sb[:, :],
                    scalar1=si_sb[:, t:t + 1], scalar2=1.0, op0=eq, op1=sub)

                scr = scrpool.tile([CHUNK, NCLS], bf16)
                lg = lgpool.tile([CHUNK, 1], f32)
                nc.vector.affine_mul_reduce(
                    out=scr[:, :], accum_out=lg[:, :], in0=oht[:, :],
                    in1=py[:, 0:NCLS], scale=1.0, bias=0.0)
                yb = ypool.tile([CHUNK, 54], bf16)
                nc.vector.tensor_copy(out=yb[:, :], in_=py[:, NCLS:MOV])
                ys[t] = yb

                et = etpool.tile([CHUNK, SLOTS], bf16)
                nc.scalar.activation(et[:, :], ind[:, :],
                                     mybir.ActivationFunctionType.Exp,
                                     bias=lg[:, 0:1], scale=50.0)
                ets[t] = et

                if t >= LAG:
                    emit_bag(t - LAG)
            for t2 in range(nchunk - LAG, nchunk):
                emit_bag(t2)

            nc.scalar.dma_start(out=ut_d[:, :], in_=ut_sb[:, :])

    nc.compile()
    return nc


def _prepare(x, rel_weight, att_weight, bias, attention_query, scope):
    import ml_dtypes
    x = np.asarray(x, dtype=np.float32)
    rel_weight = np.asarray(rel_weight, dtype=np.float32)
    att_weight = np.asarray(att_weight, dtype=np.float32)
    bias = np.asarray(bias, dtype=np.float32)
    q = np.asarray(attention_query).astype(np.int64)
    scope = np.asarray(scope).astype(np.int64)

    nsent = x.shape[0]
    nbags = len(scope) - 1
    score = nsent // NCORES
    seg = np.searchsorted(scope, np.arange(nsent), side='right') - 1

    packs = [_pack_core(scope, seg, c * score, (c + 1) * score)
             for c in range(NCORES)]
    nchunk = max((len(p[0]) + CHUNK - 1) // CHUNK for p in packs)
    lcm = (GCH * DMAB) // np.gcd(GCH, DMAB)
    nchunk = (nchunk + lcm - 1) // lcm * lcm
    S = nchunk * CHUNK
    ngroups = nchunk // GCH

    # [cw | rel | unit] blocked per dtile: [116, 6*107]; row 115 is the
    # constant-ones row of xt, col 106 of dtile 0 routes it to PY[:,106]=1.
    cw = att_weight * rel_weight
    M = np.concatenate([cw, rel_weight], axis=0)        # [106, 690]
    cwrel = np.zeros((XR, NDT * MOV), np.float32)
    for j in range(NDT):
        cwrel[0:DW, j * MOV:j * MOV + 2 * NCLS] = M[:, j * DW:(j + 1) * DW].T
    cwrel[DW, 0 * MOV + 2 * NCLS] = 1.0
    cwrel = cwrel.astype(ml_dtypes.bfloat16)
    iot = np.ascontiguousarray(np.broadcast_to(
        np.arange(SLOTS, dtype=np.float32), (CHUNK, SLOTS))
    ).astype(ml_dtypes.bfloat16)

    in_maps, frag2bag = [], []
    for c in range(NCORES):
        rows, slots, f2b = packs[c]
        idx = np.full(S, -1, np.int64)
        idx[:len(rows)] = rows
        sl = np.full(S, -1, np.int64)
        sl[:len(slots)] = slots
        valid = idx >= 0

        xp = np.zeros((S, DIM), ml_dtypes.bfloat16)
        xp[valid] = x[idx[valid]].astype(ml_dtypes.bfloat16)
        # -> [115, nchunk, 6, 128] -> [115, nchunk*768]; append ones row
        xt = np.empty((XR, nchunk * NDT * CHUNK), ml_dtypes.bfloat16)
        xt[0:DW] = np.ascontiguousarray(
            xp.reshape(nchunk, CHUNK, NDT, DW).transpose(3, 0, 2, 1)
        ).reshape(DW, nchunk * NDT * CHUNK)
        xt[DW] = 1.0

        qp = np.full(S, -1.0, np.float32)
        qp[valid] = q[idx[valid]]
        si = sl.astype(np.float32)

        f2b_arr = np.full((ngroups, SLOTS), -1, np.int64)
        for g, m in enumerate(f2b):
            for b, s_ in m.items():
                f2b_arr[g, s_] = b
        frag2bag.append(f2b_arr)
        in_maps.append({
            "xt": xt,
            "qi": np.ascontiguousarray(qp.reshape(nchunk, CHUNK).T),
            "si": np.ascontiguousarray(si.reshape(nchunk, CHUNK).T),
            "io": iot,
            "cwrel": cwrel,
        })
    return in_maps, frag2bag, nchunk, nbags, bias


def _assemble(tables, frag2bag, nchunk, nbags, bias):
    ngroups = nchunk // GCH
    num = np.zeros((nbags, NCLS))
    den = np.zeros(nbags)
    for c in range(NCORES):
        ut = np.asarray(tables[c], dtype=np.float64).reshape(
            SLOTS, ngroups, 54).transpose(1, 0, 2)   # [g, slot, 54]
        fb = frag2bag[c].ravel()
        U = ut.reshape(ngroups * SLOTS, 54)
        v = fb >= 0
        np.add.at(num, fb[v], U[v, 0:53])
        np.add.at(den, fb[v], U[v, 53])
    return (num / den[:, None] + bias[None, :]).astype(np.float32)


def kernel(x, rel_weight, att_weight, bias, attention_query, scope):
    from concourse.bass_utils import run_bass_kernel_spmd

    in_maps, frag2bag, nchunk, nbags, b = _prepare(
        x, rel_weight, att_weight, bias, attention_query, scope)
    if nchunk not in _cache:
        _cache[nchunk] = _build_module(nchunk)
    nc = _cache[nchunk]
    res = run_bass_kernel_spmd(nc, in_maps, list(range(NCORES)))
    tables = [res.results[c]["ut"] for c in range(NCORES)]
    return _assemble(tables, frag2bag, nchunk, nbags, b)
